# revision 33
# baseline (speedup 1.0000x reference)
"""Trainium2 Bass kernel for nn_Mismatch_loss (weighted per-channel MSE loss).

Contract: kernel(**inputs) takes FULL fp32 inputs (net_out, target,
max_positiones of shape [8, 16, 384, 384]) and returns the FULL scalar
output, distributing work across 8 NeuronCores internally.

Sharding: data-parallel over batch — core b processes image b.

Math per (b, c) channel (spatial reductions over 384*384 = HW elements):
    d   = t - n
    d2  = d * d
    S1  = sum(t)        (= d1 in the reference)
    S2  = sum(d2)       (= m1 + m2)
    S3  = sum(d2 * t)   (= m1)
    loss = ALPHA*S3/(S1+eps) + (1-ALPHA)*(S2-S3)/(HW-S1+eps)
The tiny [B, C] -> scalar finalization (active-mask, count of nonzero
losses, means) runs on host from the gathered per-channel sums.

Device layout per core: channel c is a [128, 1152] tile (partition-major
split of the 147456 spatial elements). Engines:
  - DVE: d = t - n, p = d2 * t      (fp16 tensor_tensor, 2x mode)
  - ACT: d2 = Square(d) with accum_out -> per-partition sum(d2) columns
  - PE : per-channel column sums of t and p via one-hot fp16 weights,
         accumulated across chunks/channels into PSUM [16, 512]
  - fp32 accumulation everywhere (PSUM / accum_out are fp32)

Inputs are cast to fp16 on host before upload: halves HBM traffic (the
kernel is DMA-bound) at ~1e-5 relative error on the final scalar.

max_positiones is only consulted when a channel of target is exactly
all-zero (cannot happen for this problem's random-uniform inputs); that
case is handled exactly on host without shipping the tensor to devices.
"""

import os
import sys

import numpy as np

for _p in ("/opt/trn_rl_repo", "/root/.axon_site/_ro/trn_rl_repo"):
    if os.path.isdir(_p) and _p not in sys.path:
        sys.path.append(_p)

B, C, H, W = 8, 16, 384, 384
HWE = H * W          # 147456 spatial elements per channel
P = 128              # SBUF partitions
F = HWE // P         # 1152 elements per partition per channel
MACRO = 4            # channels per macro tile (per DMA)
N_MACRO = C // MACRO
CHUNKS = (512, 512, 128)   # PE matmul free-dim chunking of F
SMOOTH = 1e-6
ALPHA = 0.05

_CACHE = {}


def _build_bass():
    import concourse.bacc as bacc
    import concourse.mybir as mybir
    from concourse.tile import TileContext

    f16 = mybir.dt.float16
    f32 = mybir.dt.float32
    Alu = mybir.AluOpType

    # num_devices=1: the 8 cores run fully independent SPMD instances (no
    # collectives), so no cross-core barriers are needed.
    nc = bacc.Bacc(
        "TRN2", target_bir_lowering=False, debug=False, num_devices=1
    )
    t_in = nc.dram_tensor("t_in", [C, P, F], f16, kind="ExternalInput")
    n_in = nc.dram_tensor("n_in", [C, P, F], f16, kind="ExternalInput")
    # oneh[p, c, m] = 1.0 where m == c: stationary weights routing channel
    # c's column sums to PSUM partition c.
    oneh_in = nc.dram_tensor("oneh", [P, C, 16], f16, kind="ExternalInput")
    out_s13 = nc.dram_tensor("out_s13", [16, 2], f32, kind="ExternalOutput")
    out_acc2 = nc.dram_tensor("out_acc2", [P, C], f32, kind="ExternalOutput")

    with TileContext(nc) as tc:
        with (
            tc.tile_pool(name="io", bufs=N_MACRO) as io_pool,
            tc.tile_pool(name="cpool", bufs=6) as ch_pool,
            tc.tile_pool(name="consts", bufs=1) as const_pool,
            tc.tile_pool(name="accs", bufs=1) as acc_pool,
            tc.tile_pool(name="ps", bufs=1, space="PSUM") as psum_pool,
        ):
            oneh = const_pool.tile([P, C, 16], f16)
            nc.sync.dma_start(oneh, oneh_in.ap())
            acc2 = acc_pool.tile([P, C], f32)     # per-partition sum(d2), col c
            s13 = acc_pool.tile([16, 2], f32)
            psum1 = psum_pool.tile([16, 512], f32)  # sum(t) partials
            psum3 = psum_pool.tile([16, 512], f32)  # sum(d2*t) partials

            # Prefetch everything: all input DMAs issue up front, so the
            # SDMA engines stream continuously at HBM rate.
            t_tiles, n_tiles = [], []
            for m in range(N_MACRO):
                c0 = m * MACRO
                t_t = io_pool.tile([P, MACRO, F], f16, tag="t")
                n_t = io_pool.tile([P, MACRO, F], f16, tag="n")
                nc.sync.dma_start(
                    t_t, t_in.ap()[c0 : c0 + MACRO].rearrange("c p f -> p c f")
                )
                nc.sync.dma_start(
                    n_t, n_in.ap()[c0 : c0 + MACRO].rearrange("c p f -> p c f")
                )
                t_tiles.append(t_t)
                n_tiles.append(n_t)

            for m in range(N_MACRO):
                c0 = m * MACRO
                t_t, n_t = t_tiles[m], n_tiles[m]
                for lc in range(MACRO):
                    c = c0 + lc
                    d_c = ch_pool.tile([P, F], f16, tag="d")
                    nc.vector.tensor_tensor(
                        d_c, t_t[:, lc, :], n_t[:, lc, :], Alu.subtract
                    )
                    d2_c = ch_pool.tile([P, F], f16, tag="d2")
                    nc.scalar.activation(
                        d2_c,
                        d_c,
                        mybir.ActivationFunctionType.Square,
                        accum_out=acc2[:, c : c + 1],
                    )
                    p_c = ch_pool.tile([P, F], f16, tag="p")
                    nc.vector.tensor_tensor(p_c, d2_c, t_t[:, lc, :], Alu.mult)
                    w = oneh[:, c, :]
                    off = 0
                    for wdt in CHUNKS:
                        first = c == 0 and off == 0
                        last = c == C - 1 and off + wdt == F
                        nc.tensor.matmul(
                            psum1[:, 0:wdt],
                            lhsT=w,
                            rhs=t_t[:, lc, off : off + wdt],
                            start=first,
                            stop=last,
                            skip_group_check=True,
                        )
                        nc.tensor.matmul(
                            psum3[:, 0:wdt],
                            lhsT=w,
                            rhs=p_c[:, off : off + wdt],
                            start=first,
                            stop=last,
                            skip_group_check=True,
                        )
                        off += wdt

            nc.vector.tensor_reduce(
                s13[:, 0:1], psum1, axis=mybir.AxisListType.X, op=Alu.add
            )
            nc.vector.tensor_reduce(
                s13[:, 1:2], psum3, axis=mybir.AxisListType.X, op=Alu.add
            )
            nc.sync.dma_start(out_s13.ap(), s13)
            nc.sync.dma_start(out_acc2.ap(), acc2)

    nc.compile()
    return nc


def _build_bass_raw():
    """Hand-scheduled raw-bass version: same pipeline as the Tile build but
    with manual semaphores and a minimal end-of-kernel protocol, avoiding
    Tile's ~15us of preamble/postamble barriers.

    Engine programs (per core):
      SP  : all input DMAs up front (2-channel granularity), output DMAs
            gated on completion sems.
      DVE : d_c = t_c - n_c and p_c = d2_c * t_c (fp16 2x), subs running
            3 channels ahead of muls; final PSUM->[16,1] reductions.
      ACT : d2_c = Square(d_c) with fused per-partition accumulation.
      PE  : per-channel column sums of t and p into PSUM via one-hot
            weights, t-matmuls leading p-matmuls by one macro.
    """
    import concourse.bass as bass
    import concourse.mybir as mybir

    f16 = mybir.dt.float16
    f32 = mybir.dt.float32
    Alu = mybir.AluOpType
    Act = mybir.ActivationFunctionType

    # Channel groups per input DMA: first channels in 1-channel DMAs so
    # compute starts as early as possible, last channels in 1-channel DMAs
    # so the end-of-stream dependency tail is short, 2-channel in between.
    GROUPS = [[0], [1], [2], [3]] + [[c, c + 1] for c in range(4, C, 2)]
    NG = len(GROUPS)
    grp_of = {}
    for g, chans in enumerate(GROUPS):
        for c in chans:
            grp_of[c] = g
    RING = 4                     # d/d2/p ring depth (channels in flight)

    nc = bass.Bass("TRN2", target_bir_lowering=False, debug=False, num_devices=1)
    t_in = nc.dram_tensor("t_in", [C, P, F], f16, kind="ExternalInput")
    n_in = nc.dram_tensor("n_in", [C, P, F], f16, kind="ExternalInput")
    # Single merged output: cols 0..15 = per-partition sum(d2) (acc2);
    # [0:16, 16] = per-channel sum(t); [0:16, 17] = per-channel sum(d2*t).
    out_all = nc.dram_tensor("out_all", [P, C + 2], f32, kind="ExternalOutput")

    from contextlib import ExitStack

    with ExitStack() as ctx:
        ctx.enter_context(nc.cleanup_on_exit())
        sb = lambda name, shape, dtype: ctx.enter_context(  # noqa: E731
            nc.sbuf_tensor(name, shape, dtype)
        )
        t_sb = {}
        n_sb = {}
        for g, chans in enumerate(GROUPS):
            t_sb[g] = sb(f"t_sb{g}", [P, len(chans), F], f16)
            n_sb[g] = sb(f"n_sb{g}", [P, len(chans), F], f16)
        d_sb = [sb(f"d_sb{k}", [P, F], f16) for k in range(RING)]
        d2_sb = [sb(f"d2_sb{k}", [P, F], f16) for k in range(RING)]
        p_sb = [sb(f"p_sb{k}", [P, F], f16) for k in range(RING)]
        oneh = sb("oneh_sb", [P, C, 16], f16)
        outb = sb("outb_sb", [P, C + 2], f32)
        scratch = sb("scratch_sb", [P, 1], f16)
        red_scr = sb("red_scr_sb", [16, 512], f32)
        psum1 = ctx.enter_context(nc.psum_tensor("psum1", [16, 512], f32))
        psum3 = ctx.enter_context(nc.psum_tensor("psum3", [16, 512], f32))

        sem = nc.alloc_semaphore
        s_t = [sem(f"s_t{g}") for g in range(NG)]
        s_n = [sem(f"s_n{g}") for g in range(NG)]
        s_oneh = sem("s_oneh")
        s_d = sem("s_d")      # subs completed
        s_sq = sem("s_sq")    # squares completed
        s_p = sem("s_p")      # muls completed
        s_pet = sem("s_pet")  # PE t-matmul channel groups completed
        s_pep = sem("s_pep")  # PE p-matmul channel groups completed
        s_red = sem("s_red")  # final reductions completed
        s_out = sem("s_out")  # output DMA completed

        def t_ap(c):
            g = grp_of[c]
            return t_sb[g][:, c - GROUPS[g][0], :]

        def n_ap(c):
            g = grp_of[c]
            return n_sb[g][:, c - GROUPS[g][0], :]

        # ---- GPSIMD: build one-hot weights on device (no DMA needed) ----
        nc.gpsimd.memset(oneh[:, :, :], 0.0)
        for c in range(C):
            ms = nc.gpsimd.memset(oneh[:, c, c : c + 1], 1.0)
        ms.then_inc(s_oneh, 1)

        # ---- SP: DMAs ----
        def in_dma(dst, src, chans, s):
            c0 = chans[0]
            nc.sync.dma_start(
                dst[:, :, :],
                src[c0 : c0 + len(chans)].rearrange("c p f -> p c f"),
            ).then_inc(s, 16)

        for g in range(NG):
            in_dma(t_sb[g], t_in.ap(), GROUPS[g], s_t[g])
            in_dma(n_sb[g], n_in.ap(), GROUPS[g], s_n[g])
        # acc2 columns ship as soon as the squares finish (overlaps the
        # final muls/matmuls); the tiny reduction outputs ship last.
        nc.sync.wait_ge(s_sq, C)
        nc.sync.dma_start(
            out_all.ap()[:, 0:C], outb[:, 0:C]
        ).then_inc(s_out, 16)
        nc.sync.wait_ge(s_red, 2)
        nc.sync.dma_start(
            out_all.ap()[0:16, C : C + 2], outb[0:16, C : C + 2]
        ).then_inc(s_out, 16)
        nc.sync.wait_ge(s_out, 32)

        # ---- DVE: subs (3 channels ahead) and muls ----
        def emit_sub(c):
            g = grp_of[c]
            if c == GROUPS[g][0]:
                nc.vector.wait_ge(s_t[g], 16)
                nc.vector.wait_ge(s_n[g], 16)
            nc.vector.tensor_tensor(
                d_sb[c % RING][:, :], t_ap(c), n_ap(c), Alu.subtract
            ).then_inc(s_d, 1)

        def emit_mul(j):
            nc.vector.wait_ge(s_sq, j + 1)
            if j >= RING:
                nc.vector.wait_ge(s_pep, j - (RING - 1))
            nc.vector.tensor_tensor(
                p_sb[j % RING][:, :], d2_sb[j % RING][:, :], t_ap(j), Alu.mult
            ).then_inc(s_p, 1)

        SKEW = 2
        for i in range(C + SKEW):
            if i < C:
                emit_sub(i)
            if i - SKEW >= 0:
                emit_mul(i - SKEW)

        # ---- ACT: squares with fused per-partition accumulation ----
        # Dummy activation first: pulls the one-time ACT_TABLE_LOAD
        # (~1.3us) off the critical path of the first real square.
        nc.scalar.activation(scratch[:, :], scratch[:, :], Act.Square)
        for c in range(C):
            nc.scalar.wait_ge(s_d, c + 1)
            if c >= RING:
                nc.scalar.wait_ge(s_p, c - (RING - 1))
            nc.scalar.activation(
                d2_sb[c % RING][:, :],
                d_sb[c % RING][:, :],
                Act.Square,
                accum_out=outb[:, c : c + 1],
            ).then_inc(s_sq, 1)
        # Final PSUM -> [16,1] reductions via Copy+accum_out (ACT is idle
        # by now and sits closer to PSUM than the vector engine).
        nc.scalar.wait_ge(s_pet, C)
        nc.scalar.activation(
            red_scr[:, :], psum1[:, :], Act.Copy,
            accum_out=outb[0:16, C : C + 1],
        ).then_inc(s_red, 1)
        nc.scalar.wait_ge(s_pep, C)
        nc.scalar.activation(
            red_scr[:, :], psum3[:, :], Act.Copy,
            accum_out=outb[0:16, C + 1 : C + 2],
        ).then_inc(s_red, 1)

        # ---- PE: one-hot column-sum matmuls; t leads p by one group ----
        def emit_t_mms(g):
            nc.tensor.wait_ge(s_t[g], 16)
            for lc, c in enumerate(GROUPS[g]):
                w = oneh[:, c, :]
                off = 0
                for wdt in CHUNKS:
                    mm = nc.tensor.matmul(
                        psum1[:, 0:wdt],
                        lhsT=w,
                        rhs=t_sb[g][:, lc, off : off + wdt],
                        start=(c == 0 and off == 0),
                        stop=(c == C - 1 and off + wdt == F),
                        skip_group_check=True,
                    )
                    off += wdt
                mm.then_inc(s_pet, 1)

        def emit_p_mms(c):
            nc.tensor.wait_ge(s_p, c + 1)
            w = oneh[:, c, :]
            off = 0
            for wdt in CHUNKS:
                mm = nc.tensor.matmul(
                    psum3[:, 0:wdt],
                    lhsT=w,
                    rhs=p_sb[c % RING][:, off : off + wdt],
                    start=(c == 0 and off == 0),
                    stop=(c == C - 1 and off + wdt == F),
                    skip_group_check=True,
                )
                off += wdt
            mm.then_inc(s_pep, 1)

        nc.tensor.wait_ge(s_oneh, 1)
        emit_t_mms(0)
        done_p = 0
        for g in range(1, NG):
            emit_t_mms(g)
            for c in GROUPS[g - 1]:
                emit_p_mms(c)
                done_p = c
        for c in range(done_p + 1, C):
            emit_p_mms(c)

        nc.all_engine_barrier()

    return nc


def _get_nc():
    impl = os.environ.get("BASS_LOSS_IMPL", "raw")
    key = f"nc_{impl}"
    if key not in _CACHE:
        _CACHE[key] = _build_bass_raw() if impl == "raw" else _build_bass()
    return _CACHE[key]


def kernel(net_out, target, max_positiones):
    from concourse import bass_utils

    nc = _get_nc()

    t16 = np.asarray(target, dtype=np.float16).reshape(B, C, P, F)
    n16 = np.asarray(net_out, dtype=np.float16).reshape(B, C, P, F)

    expected_inputs = set()
    import concourse.mybir as mybir

    for alloc in nc.m.functions[0].allocations:
        if (
            isinstance(alloc, mybir.MemoryLocationSet)
            and alloc.kind == "ExternalInput"
        ):
            expected_inputs.add(alloc.memorylocations[0].name)

    def make_map(b):
        m = {"t_in": t16[b], "n_in": n16[b]}
        if "oneh" in expected_inputs:
            oneh = np.zeros((P, C, 16), dtype=np.float16)
            for c in range(C):
                oneh[:, c, c] = 1.0
            m["oneh"] = oneh
        return m

    in_maps = [make_map(b) for b in range(B)]
    # The axon terminal occasionally reports the accelerator unrecoverable
    # on the first touch after a previous process ran a NEFF; the failed
    # attempt itself triggers recovery, so retry.
    last_err = None
    for _attempt in range(4):
        try:
            res = bass_utils.run_bass_kernel_spmd(
                nc, in_maps, core_ids=list(range(8))
            )
            break
        except Exception as e:  # noqa: BLE001
            last_err = e
            import time as _time

            _time.sleep(2.0)
    else:
        raise last_err

    S1 = np.empty((B, C), np.float64)
    S2 = np.empty((B, C), np.float64)
    S3 = np.empty((B, C), np.float64)
    for b in range(B):
        r = res.results[b]
        if "out_all" in r:
            out = r["out_all"].astype(np.float64)
            S1[b] = out[:16, C]
            S3[b] = out[:16, C + 1]
            S2[b] = out[:, :C].sum(axis=0)
        else:
            s13 = r["out_s13"].astype(np.float64)
            S1[b] = s13[:, 0]
            S3[b] = s13[:, 1]
            S2[b] = r["out_acc2"].astype(np.float64).sum(axis=0)

    m1, m2, d1 = S3, S2 - S3, S1
    d2n = float(HWE) - d1
    loss = ALPHA * m1 / (d1 + SMOOTH) + (1.0 - ALPHA) * m2 / (d2n + SMOOTH)

    # active-mask: S1 != 0 implies max(target[b,c]) != 0 for non-negative
    # targets; the S1 == 0 corner is resolved exactly on host.
    active = S1 != 0.0
    for b, c in zip(*np.nonzero(~active)):
        mt = np.max(target[b, c])
        mmp = np.max(max_positiones[b, c])
        active[b, c] = not (mt == 0.0 and mmp == 0.0)

    losses = np.where(active, loss, 0.0)
    count = (losses != 0.0).sum(axis=1).astype(np.float64)
    img_losses = losses.sum(axis=1) / count
    return np.float32(img_losses.mean())


# revision 37
# speedup vs baseline: 1.0665x; 1.0665x over previous
"""Trainium2 Bass kernel for nn_Mismatch_loss (weighted per-channel MSE loss).

Contract: kernel(**inputs) takes FULL fp32 inputs (net_out, target,
max_positiones of shape [8, 16, 384, 384]) and returns the FULL scalar
output, distributing work across 8 NeuronCores internally.

Sharding: data-parallel over batch — core b processes image b.

Math per (b, c) channel (spatial reductions over 384*384 = HW elements):
    d   = t - n
    d2  = d * d
    S1  = sum(t)        (= d1 in the reference)
    S2  = sum(d2)       (= m1 + m2)
    S3  = sum(d2 * t)   (= m1)
    loss = ALPHA*S3/(S1+eps) + (1-ALPHA)*(S2-S3)/(HW-S1+eps)
The tiny [B, C] -> scalar finalization (active-mask, count of nonzero
losses, means) runs on host from the gathered per-channel sums.

Device layout per core: channel c is a [128, 1152] tile (partition-major
split of the 147456 spatial elements). Engines:
  - DVE: d = t - n, p = d2 * t      (fp16 tensor_tensor, 2x mode)
  - ACT: d2 = Square(d) with accum_out -> per-partition sum(d2) columns
  - PE : per-channel column sums of t and p via one-hot fp16 weights,
         accumulated across chunks/channels into PSUM [16, 512]
  - fp32 accumulation everywhere (PSUM / accum_out are fp32)

Inputs are cast to fp16 on host before upload: halves HBM traffic (the
kernel is DMA-bound) at ~1e-5 relative error on the final scalar.

max_positiones is only consulted when a channel of target is exactly
all-zero (cannot happen for this problem's random-uniform inputs); that
case is handled exactly on host without shipping the tensor to devices.
"""

import os
import sys

import numpy as np

for _p in ("/opt/trn_rl_repo", "/root/.axon_site/_ro/trn_rl_repo"):
    if os.path.isdir(_p) and _p not in sys.path:
        sys.path.append(_p)

B, C, H, W = 8, 16, 384, 384
HWE = H * W          # 147456 spatial elements per channel
P = 128              # SBUF partitions
F = HWE // P         # 1152 elements per partition per channel
MACRO = 4            # channels per macro tile (per DMA)
N_MACRO = C // MACRO
CHUNKS = (512, 512, 128)   # PE matmul free-dim chunking of F
SMOOTH = 1e-6
ALPHA = 0.05

_CACHE = {}


def _build_bass():
    import concourse.bacc as bacc
    import concourse.mybir as mybir
    from concourse.tile import TileContext

    f16 = mybir.dt.float16
    f32 = mybir.dt.float32
    Alu = mybir.AluOpType

    # num_devices=1: the 8 cores run fully independent SPMD instances (no
    # collectives), so no cross-core barriers are needed.
    nc = bacc.Bacc(
        "TRN2", target_bir_lowering=False, debug=False, num_devices=1
    )
    t_in = nc.dram_tensor("t_in", [C, P, F], f16, kind="ExternalInput")
    n_in = nc.dram_tensor("n_in", [C, P, F], f16, kind="ExternalInput")
    # oneh[p, c, m] = 1.0 where m == c: stationary weights routing channel
    # c's column sums to PSUM partition c.
    oneh_in = nc.dram_tensor("oneh", [P, C, 16], f16, kind="ExternalInput")
    out_s13 = nc.dram_tensor("out_s13", [16, 2], f32, kind="ExternalOutput")
    out_acc2 = nc.dram_tensor("out_acc2", [P, C], f32, kind="ExternalOutput")

    with TileContext(nc) as tc:
        with (
            tc.tile_pool(name="io", bufs=N_MACRO) as io_pool,
            tc.tile_pool(name="cpool", bufs=6) as ch_pool,
            tc.tile_pool(name="consts", bufs=1) as const_pool,
            tc.tile_pool(name="accs", bufs=1) as acc_pool,
            tc.tile_pool(name="ps", bufs=1, space="PSUM") as psum_pool,
        ):
            oneh = const_pool.tile([P, C, 16], f16)
            nc.sync.dma_start(oneh, oneh_in.ap())
            acc2 = acc_pool.tile([P, C], f32)     # per-partition sum(d2), col c
            s13 = acc_pool.tile([16, 2], f32)
            psum1 = psum_pool.tile([16, 512], f32)  # sum(t) partials
            psum3 = psum_pool.tile([16, 512], f32)  # sum(d2*t) partials

            # Prefetch everything: all input DMAs issue up front, so the
            # SDMA engines stream continuously at HBM rate.
            t_tiles, n_tiles = [], []
            for m in range(N_MACRO):
                c0 = m * MACRO
                t_t = io_pool.tile([P, MACRO, F], f16, tag="t")
                n_t = io_pool.tile([P, MACRO, F], f16, tag="n")
                nc.sync.dma_start(
                    t_t, t_in.ap()[c0 : c0 + MACRO].rearrange("c p f -> p c f")
                )
                nc.sync.dma_start(
                    n_t, n_in.ap()[c0 : c0 + MACRO].rearrange("c p f -> p c f")
                )
                t_tiles.append(t_t)
                n_tiles.append(n_t)

            for m in range(N_MACRO):
                c0 = m * MACRO
                t_t, n_t = t_tiles[m], n_tiles[m]
                for lc in range(MACRO):
                    c = c0 + lc
                    d_c = ch_pool.tile([P, F], f16, tag="d")
                    nc.vector.tensor_tensor(
                        d_c, t_t[:, lc, :], n_t[:, lc, :], Alu.subtract
                    )
                    d2_c = ch_pool.tile([P, F], f16, tag="d2")
                    nc.scalar.activation(
                        d2_c,
                        d_c,
                        mybir.ActivationFunctionType.Square,
                        accum_out=acc2[:, c : c + 1],
                    )
                    p_c = ch_pool.tile([P, F], f16, tag="p")
                    nc.vector.tensor_tensor(p_c, d2_c, t_t[:, lc, :], Alu.mult)
                    w = oneh[:, c, :]
                    off = 0
                    for wdt in CHUNKS:
                        first = c == 0 and off == 0
                        last = c == C - 1 and off + wdt == F
                        nc.tensor.matmul(
                            psum1[:, 0:wdt],
                            lhsT=w,
                            rhs=t_t[:, lc, off : off + wdt],
                            start=first,
                            stop=last,
                            skip_group_check=True,
                        )
                        nc.tensor.matmul(
                            psum3[:, 0:wdt],
                            lhsT=w,
                            rhs=p_c[:, off : off + wdt],
                            start=first,
                            stop=last,
                            skip_group_check=True,
                        )
                        off += wdt

            nc.vector.tensor_reduce(
                s13[:, 0:1], psum1, axis=mybir.AxisListType.X, op=Alu.add
            )
            nc.vector.tensor_reduce(
                s13[:, 1:2], psum3, axis=mybir.AxisListType.X, op=Alu.add
            )
            nc.sync.dma_start(out_s13.ap(), s13)
            nc.sync.dma_start(out_acc2.ap(), acc2)

    nc.compile()
    return nc


def _build_bass_raw():
    """Hand-scheduled raw-bass version: same pipeline as the Tile build but
    with manual semaphores and a minimal end-of-kernel protocol, avoiding
    Tile's ~15us of preamble/postamble barriers.

    Engine programs (per core):
      SP  : all input DMAs up front (2-channel granularity), output DMAs
            gated on completion sems.
      DVE : d_c = t_c - n_c and p_c = d2_c * t_c (fp16 2x), subs running
            3 channels ahead of muls; final PSUM->[16,1] reductions.
      ACT : d2_c = Square(d_c) with fused per-partition accumulation.
      PE  : per-channel column sums of t and p into PSUM via one-hot
            weights, t-matmuls leading p-matmuls by one macro.
    """
    import concourse.bass as bass
    import concourse.mybir as mybir

    f16 = mybir.dt.float16
    f32 = mybir.dt.float32
    Alu = mybir.AluOpType
    Act = mybir.ActivationFunctionType

    # Channel groups per input DMA: first channels in 1-channel DMAs so
    # compute starts as early as possible, last channels in 1-channel DMAs
    # so the end-of-stream dependency tail is short, 2-channel in between.
    n_singles = int(os.environ.get("BASS_HEAD_SINGLES", "2"))
    GROUPS = [[c] for c in range(n_singles)] + [
        [c, c + 1] for c in range(n_singles, C, 2)
    ]
    NG = len(GROUPS)
    grp_of = {}
    for g, chans in enumerate(GROUPS):
        for c in chans:
            grp_of[c] = g
    RING = 4                     # d/d2/p ring depth (channels in flight)

    nc = bass.Bass("TRN2", target_bir_lowering=False, debug=False, num_devices=1)
    t_in = nc.dram_tensor("t_in", [C, P, F], f16, kind="ExternalInput")
    n_in = nc.dram_tensor("n_in", [C, P, F], f16, kind="ExternalInput")
    # Single merged output: cols 0..15 = per-partition sum(d2) (acc2);
    # [0:16, 16] = per-channel sum(t); [0:16, 17] = per-channel sum(d2*t).
    out_all = nc.dram_tensor("out_all", [P, C + 2], f32, kind="ExternalOutput")

    from contextlib import ExitStack

    with ExitStack() as ctx:
        ctx.enter_context(nc.cleanup_on_exit())
        sb = lambda name, shape, dtype: ctx.enter_context(  # noqa: E731
            nc.sbuf_tensor(name, shape, dtype)
        )
        t_sb = {}
        n_sb = {}
        for g, chans in enumerate(GROUPS):
            t_sb[g] = sb(f"t_sb{g}", [P, len(chans), F], f16)
            n_sb[g] = sb(f"n_sb{g}", [P, len(chans), F], f16)
        d_sb = [sb(f"d_sb{k}", [P, F], f16) for k in range(RING)]
        d2_sb = [sb(f"d2_sb{k}", [P, F], f16) for k in range(RING)]
        p_sb = [sb(f"p_sb{k}", [P, F], f16) for k in range(RING)]
        oneh = sb("oneh_sb", [P, C, 16], f16)
        outb = sb("outb_sb", [P, C + 2], f32)
        scratch = sb("scratch_sb", [P, 1], f16)
        red_scr = sb("red_scr_sb", [16, 512], f32)
        psum1 = ctx.enter_context(nc.psum_tensor("psum1", [16, 512], f32))
        psum3 = ctx.enter_context(nc.psum_tensor("psum3", [16, 512], f32))

        sem = nc.alloc_semaphore
        s_t = [sem(f"s_t{g}") for g in range(NG)]
        s_n = [sem(f"s_n{g}") for g in range(NG)]
        s_oneh = sem("s_oneh")
        s_d = sem("s_d")      # subs completed
        s_sq = sem("s_sq")    # squares completed
        s_p = sem("s_p")      # muls completed
        s_pet = sem("s_pet")  # PE t-matmul channel groups completed
        s_pep = sem("s_pep")  # PE p-matmul channel groups completed
        s_red = sem("s_red")  # final reductions completed
        s_out = sem("s_out")  # output DMA completed

        def t_ap(c):
            g = grp_of[c]
            return t_sb[g][:, c - GROUPS[g][0], :]

        def n_ap(c):
            g = grp_of[c]
            return n_sb[g][:, c - GROUPS[g][0], :]

        # ---- GPSIMD: build one-hot weights on device (no DMA needed) ----
        nc.gpsimd.memset(oneh[:, :, :], 0.0)
        for c in range(C):
            ms = nc.gpsimd.memset(oneh[:, c, c : c + 1], 1.0)
        ms.then_inc(s_oneh, 1)

        # ---- SP: DMAs ----
        def in_dma(dst, src, chans, s):
            c0 = chans[0]
            nc.sync.dma_start(
                dst[:, :, :],
                src[c0 : c0 + len(chans)].rearrange("c p f -> p c f"),
            ).then_inc(s, 16)

        for g in range(NG):
            in_dma(t_sb[g], t_in.ap(), GROUPS[g], s_t[g])
            in_dma(n_sb[g], n_in.ap(), GROUPS[g], s_n[g])
        # acc2 columns ship as soon as the squares finish (overlaps the
        # final muls/matmuls); the tiny reduction outputs ship last.
        nc.sync.wait_ge(s_sq, C)
        nc.sync.dma_start(
            out_all.ap()[:, 0:C], outb[:, 0:C]
        ).then_inc(s_out, 16)
        nc.sync.wait_ge(s_red, 2)
        nc.sync.dma_start(
            out_all.ap()[0:16, C : C + 2], outb[0:16, C : C + 2]
        ).then_inc(s_out, 16)
        nc.sync.wait_ge(s_out, 32)

        # ---- DVE: subs (3 channels ahead) and muls ----
        def emit_sub(c):
            g = grp_of[c]
            if c == GROUPS[g][0]:
                nc.vector.wait_ge(s_t[g], 16)
                nc.vector.wait_ge(s_n[g], 16)
            nc.vector.tensor_tensor(
                d_sb[c % RING][:, :], t_ap(c), n_ap(c), Alu.subtract
            ).then_inc(s_d, 1)

        def emit_mul(j):
            nc.vector.wait_ge(s_sq, j + 1)
            if j >= RING:
                nc.vector.wait_ge(s_pep, j - (RING - 1))
            nc.vector.tensor_tensor(
                p_sb[j % RING][:, :], d2_sb[j % RING][:, :], t_ap(j), Alu.mult
            ).then_inc(s_p, 1)

        SKEW = 2
        for i in range(C + SKEW):
            if i < C:
                emit_sub(i)
            if i - SKEW >= 0:
                emit_mul(i - SKEW)

        # ---- ACT: squares with fused per-partition accumulation ----
        # Dummy activation first: pulls the one-time ACT_TABLE_LOAD
        # (~1.3us) off the critical path of the first real square.
        nc.scalar.activation(scratch[:, :], scratch[:, :], Act.Square)
        for c in range(C):
            nc.scalar.wait_ge(s_d, c + 1)
            if c >= RING:
                nc.scalar.wait_ge(s_p, c - (RING - 1))
            nc.scalar.activation(
                d2_sb[c % RING][:, :],
                d_sb[c % RING][:, :],
                Act.Square,
                accum_out=outb[:, c : c + 1],
            ).then_inc(s_sq, 1)
        # Final PSUM -> [16,1] reductions.
        if os.environ.get("BASS_REDUCE_ENGINE", "dve") == "act":
            nc.scalar.wait_ge(s_pet, C)
            nc.scalar.activation(
                red_scr[:, :], psum1[:, :], Act.Copy,
                accum_out=outb[0:16, C : C + 1],
            ).then_inc(s_red, 1)
            nc.scalar.wait_ge(s_pep, C)
            nc.scalar.activation(
                red_scr[:, :], psum3[:, :], Act.Copy,
                accum_out=outb[0:16, C + 1 : C + 2],
            ).then_inc(s_red, 1)
        else:
            nc.vector.wait_ge(s_pet, C)
            nc.vector.tensor_reduce(
                outb[0:16, C : C + 1], psum1[:, :],
                axis=mybir.AxisListType.X, op=Alu.add,
            ).then_inc(s_red, 1)
            nc.vector.wait_ge(s_pep, C)
            nc.vector.tensor_reduce(
                outb[0:16, C + 1 : C + 2], psum3[:, :],
                axis=mybir.AxisListType.X, op=Alu.add,
            ).then_inc(s_red, 1)

        # ---- PE: one-hot column-sum matmuls; t leads p by one group ----
        def emit_t_mms(g):
            nc.tensor.wait_ge(s_t[g], 16)
            for lc, c in enumerate(GROUPS[g]):
                w = oneh[:, c, :]
                off = 0
                for wdt in CHUNKS:
                    mm = nc.tensor.matmul(
                        psum1[:, 0:wdt],
                        lhsT=w,
                        rhs=t_sb[g][:, lc, off : off + wdt],
                        start=(c == 0 and off == 0),
                        stop=(c == C - 1 and off + wdt == F),
                        skip_group_check=True,
                    )
                    off += wdt
                mm.then_inc(s_pet, 1)

        def emit_p_mms(c):
            nc.tensor.wait_ge(s_p, c + 1)
            w = oneh[:, c, :]
            off = 0
            for wdt in CHUNKS:
                mm = nc.tensor.matmul(
                    psum3[:, 0:wdt],
                    lhsT=w,
                    rhs=p_sb[c % RING][:, off : off + wdt],
                    start=(c == 0 and off == 0),
                    stop=(c == C - 1 and off + wdt == F),
                    skip_group_check=True,
                )
                off += wdt
            mm.then_inc(s_pep, 1)

        nc.tensor.wait_ge(s_oneh, 1)
        emit_t_mms(0)
        done_p = 0
        for g in range(1, NG):
            emit_t_mms(g)
            for c in GROUPS[g - 1]:
                emit_p_mms(c)
                done_p = c
        for c in range(done_p + 1, C):
            emit_p_mms(c)

        nc.all_engine_barrier()

    return nc


def _get_nc():
    impl = os.environ.get("BASS_LOSS_IMPL", "raw")
    key = "nc_{}_{}_{}".format(
        impl,
        os.environ.get("BASS_HEAD_SINGLES", "2"),
        os.environ.get("BASS_REDUCE_ENGINE", "dve"),
    )
    if key not in _CACHE:
        _CACHE[key] = _build_bass_raw() if impl == "raw" else _build_bass()
    return _CACHE[key]


def kernel(net_out, target, max_positiones):
    from concourse import bass_utils

    nc = _get_nc()

    t16 = np.asarray(target, dtype=np.float16).reshape(B, C, P, F)
    n16 = np.asarray(net_out, dtype=np.float16).reshape(B, C, P, F)

    expected_inputs = set()
    import concourse.mybir as mybir

    for alloc in nc.m.functions[0].allocations:
        if (
            isinstance(alloc, mybir.MemoryLocationSet)
            and alloc.kind == "ExternalInput"
        ):
            expected_inputs.add(alloc.memorylocations[0].name)

    def make_map(b):
        m = {"t_in": t16[b], "n_in": n16[b]}
        if "oneh" in expected_inputs:
            oneh = np.zeros((P, C, 16), dtype=np.float16)
            for c in range(C):
                oneh[:, c, c] = 1.0
            m["oneh"] = oneh
        return m

    in_maps = [make_map(b) for b in range(B)]
    # The axon terminal occasionally reports the accelerator unrecoverable
    # on the first touch after a previous process ran a NEFF. The failed
    # attempt triggers recovery terminal-side, but the local PJRT client
    # stays poisoned — tear it down between retries.
    last_err = None
    for _attempt in range(4):
        try:
            res = bass_utils.run_bass_kernel_spmd(
                nc, in_maps, core_ids=list(range(8))
            )
            break
        except Exception as e:  # noqa: BLE001
            last_err = e
            import time as _time

            _time.sleep(3.0)
            try:
                import jax

                jax.clear_caches()
                jax.extend.backend.clear_backends()
            except Exception:  # noqa: BLE001
                pass
            _time.sleep(2.0)
    else:
        raise last_err

    S1 = np.empty((B, C), np.float64)
    S2 = np.empty((B, C), np.float64)
    S3 = np.empty((B, C), np.float64)
    for b in range(B):
        r = res.results[b]
        if "out_all" in r:
            out = r["out_all"].astype(np.float64)
            S1[b] = out[:16, C]
            S3[b] = out[:16, C + 1]
            S2[b] = out[:, :C].sum(axis=0)
        else:
            s13 = r["out_s13"].astype(np.float64)
            S1[b] = s13[:, 0]
            S3[b] = s13[:, 1]
            S2[b] = r["out_acc2"].astype(np.float64).sum(axis=0)

    m1, m2, d1 = S3, S2 - S3, S1
    d2n = float(HWE) - d1
    loss = ALPHA * m1 / (d1 + SMOOTH) + (1.0 - ALPHA) * m2 / (d2n + SMOOTH)

    # active-mask: S1 != 0 implies max(target[b,c]) != 0 for non-negative
    # targets; the S1 == 0 corner is resolved exactly on host.
    active = S1 != 0.0
    for b, c in zip(*np.nonzero(~active)):
        mt = np.max(target[b, c])
        mmp = np.max(max_positiones[b, c])
        active[b, c] = not (mt == 0.0 and mmp == 0.0)

    losses = np.where(active, loss, 0.0)
    count = (losses != 0.0).sum(axis=1).astype(np.float64)
    img_losses = losses.sum(axis=1) / count
    return np.float32(img_losses.mean())


# revision 39
# speedup vs baseline: 1.0713x; 1.0044x over previous
"""Trainium2 Bass kernel for nn_Mismatch_loss (weighted per-channel MSE loss).

Contract: kernel(**inputs) takes FULL fp32 inputs (net_out, target,
max_positiones of shape [8, 16, 384, 384]) and returns the FULL scalar
output, distributing work across 8 NeuronCores internally.

Sharding: data-parallel over batch — core b processes image b.

Math per (b, c) channel (spatial reductions over 384*384 = HW elements):
    d   = t - n
    d2  = d * d
    S1  = sum(t)        (= d1 in the reference)
    S2  = sum(d2)       (= m1 + m2)
    S3  = sum(d2 * t)   (= m1)
    loss = ALPHA*S3/(S1+eps) + (1-ALPHA)*(S2-S3)/(HW-S1+eps)
The tiny [B, C] -> scalar finalization (active-mask, count of nonzero
losses, means) runs on host from the gathered per-channel sums.

Device layout per core: channel c is a [128, 1152] tile (partition-major
split of the 147456 spatial elements). Engines:
  - DVE: d = t - n, p = d2 * t      (fp16 tensor_tensor, 2x mode)
  - ACT: d2 = Square(d) with accum_out -> per-partition sum(d2) columns
  - PE : per-channel column sums of t and p via one-hot fp16 weights,
         accumulated across chunks/channels into PSUM [16, 512]
  - fp32 accumulation everywhere (PSUM / accum_out are fp32)

Inputs are cast to fp16 on host before upload: halves HBM traffic (the
kernel is DMA-bound) at ~1e-5 relative error on the final scalar.

max_positiones is only consulted when a channel of target is exactly
all-zero (cannot happen for this problem's random-uniform inputs); that
case is handled exactly on host without shipping the tensor to devices.
"""

import os
import sys

import numpy as np

for _p in ("/opt/trn_rl_repo", "/root/.axon_site/_ro/trn_rl_repo"):
    if os.path.isdir(_p) and _p not in sys.path:
        sys.path.append(_p)

B, C, H, W = 8, 16, 384, 384
HWE = H * W          # 147456 spatial elements per channel
P = 128              # SBUF partitions
F = HWE // P         # 1152 elements per partition per channel
MACRO = 4            # channels per macro tile (per DMA)
N_MACRO = C // MACRO
CHUNKS = (512, 512, 128)   # PE matmul free-dim chunking of F
SMOOTH = 1e-6
ALPHA = 0.05

_CACHE = {}


def _build_bass():
    import concourse.bacc as bacc
    import concourse.mybir as mybir
    from concourse.tile import TileContext

    f16 = mybir.dt.float16
    f32 = mybir.dt.float32
    Alu = mybir.AluOpType

    # num_devices=1: the 8 cores run fully independent SPMD instances (no
    # collectives), so no cross-core barriers are needed.
    nc = bacc.Bacc(
        "TRN2", target_bir_lowering=False, debug=False, num_devices=1
    )
    t_in = nc.dram_tensor("t_in", [C, P, F], f16, kind="ExternalInput")
    n_in = nc.dram_tensor("n_in", [C, P, F], f16, kind="ExternalInput")
    # oneh[p, c, m] = 1.0 where m == c: stationary weights routing channel
    # c's column sums to PSUM partition c.
    oneh_in = nc.dram_tensor("oneh", [P, C, 16], f16, kind="ExternalInput")
    out_s13 = nc.dram_tensor("out_s13", [16, 2], f32, kind="ExternalOutput")
    out_acc2 = nc.dram_tensor("out_acc2", [P, C], f32, kind="ExternalOutput")

    with TileContext(nc) as tc:
        with (
            tc.tile_pool(name="io", bufs=N_MACRO) as io_pool,
            tc.tile_pool(name="cpool", bufs=6) as ch_pool,
            tc.tile_pool(name="consts", bufs=1) as const_pool,
            tc.tile_pool(name="accs", bufs=1) as acc_pool,
            tc.tile_pool(name="ps", bufs=1, space="PSUM") as psum_pool,
        ):
            oneh = const_pool.tile([P, C, 16], f16)
            nc.sync.dma_start(oneh, oneh_in.ap())
            acc2 = acc_pool.tile([P, C], f32)     # per-partition sum(d2), col c
            s13 = acc_pool.tile([16, 2], f32)
            psum1 = psum_pool.tile([16, 512], f32)  # sum(t) partials
            psum3 = psum_pool.tile([16, 512], f32)  # sum(d2*t) partials

            # Prefetch everything: all input DMAs issue up front, so the
            # SDMA engines stream continuously at HBM rate.
            t_tiles, n_tiles = [], []
            for m in range(N_MACRO):
                c0 = m * MACRO
                t_t = io_pool.tile([P, MACRO, F], f16, tag="t")
                n_t = io_pool.tile([P, MACRO, F], f16, tag="n")
                nc.sync.dma_start(
                    t_t, t_in.ap()[c0 : c0 + MACRO].rearrange("c p f -> p c f")
                )
                nc.sync.dma_start(
                    n_t, n_in.ap()[c0 : c0 + MACRO].rearrange("c p f -> p c f")
                )
                t_tiles.append(t_t)
                n_tiles.append(n_t)

            for m in range(N_MACRO):
                c0 = m * MACRO
                t_t, n_t = t_tiles[m], n_tiles[m]
                for lc in range(MACRO):
                    c = c0 + lc
                    d_c = ch_pool.tile([P, F], f16, tag="d")
                    nc.vector.tensor_tensor(
                        d_c, t_t[:, lc, :], n_t[:, lc, :], Alu.subtract
                    )
                    d2_c = ch_pool.tile([P, F], f16, tag="d2")
                    nc.scalar.activation(
                        d2_c,
                        d_c,
                        mybir.ActivationFunctionType.Square,
                        accum_out=acc2[:, c : c + 1],
                    )
                    p_c = ch_pool.tile([P, F], f16, tag="p")
                    nc.vector.tensor_tensor(p_c, d2_c, t_t[:, lc, :], Alu.mult)
                    w = oneh[:, c, :]
                    off = 0
                    for wdt in CHUNKS:
                        first = c == 0 and off == 0
                        last = c == C - 1 and off + wdt == F
                        nc.tensor.matmul(
                            psum1[:, 0:wdt],
                            lhsT=w,
                            rhs=t_t[:, lc, off : off + wdt],
                            start=first,
                            stop=last,
                            skip_group_check=True,
                        )
                        nc.tensor.matmul(
                            psum3[:, 0:wdt],
                            lhsT=w,
                            rhs=p_c[:, off : off + wdt],
                            start=first,
                            stop=last,
                            skip_group_check=True,
                        )
                        off += wdt

            nc.vector.tensor_reduce(
                s13[:, 0:1], psum1, axis=mybir.AxisListType.X, op=Alu.add
            )
            nc.vector.tensor_reduce(
                s13[:, 1:2], psum3, axis=mybir.AxisListType.X, op=Alu.add
            )
            nc.sync.dma_start(out_s13.ap(), s13)
            nc.sync.dma_start(out_acc2.ap(), acc2)

    nc.compile()
    return nc


def _build_bass_raw():
    """Hand-scheduled raw-bass version: same pipeline as the Tile build but
    with manual semaphores and a minimal end-of-kernel protocol, avoiding
    Tile's ~15us of preamble/postamble barriers.

    Engine programs (per core):
      SP  : all input DMAs up front (2-channel granularity), output DMAs
            gated on completion sems.
      DVE : d_c = t_c - n_c and p_c = d2_c * t_c (fp16 2x), subs running
            3 channels ahead of muls; final PSUM->[16,1] reductions.
      ACT : d2_c = Square(d_c) with fused per-partition accumulation.
      PE  : per-channel column sums of t and p into PSUM via one-hot
            weights, t-matmuls leading p-matmuls by one macro.
    """
    import concourse.bass as bass
    import concourse.mybir as mybir

    f16 = mybir.dt.float16
    f32 = mybir.dt.float32
    Alu = mybir.AluOpType
    Act = mybir.ActivationFunctionType

    # Channel groups per input DMA: first channels in 1-channel DMAs so
    # compute starts as early as possible, last channels in 1-channel DMAs
    # so the end-of-stream dependency tail is short, 2-channel in between.
    n_singles = int(os.environ.get("BASS_HEAD_SINGLES", "4"))
    GROUPS = [[c] for c in range(n_singles)] + [
        [c, c + 1] for c in range(n_singles, C, 2)
    ]
    NG = len(GROUPS)
    grp_of = {}
    for g, chans in enumerate(GROUPS):
        for c in chans:
            grp_of[c] = g
    RING = 4                     # d/d2/p ring depth (channels in flight)

    nc = bass.Bass("TRN2", target_bir_lowering=False, debug=False, num_devices=1)
    t_in = nc.dram_tensor("t_in", [C, P, F], f16, kind="ExternalInput")
    n_in = nc.dram_tensor("n_in", [C, P, F], f16, kind="ExternalInput")
    # Single merged output: cols 0..15 = per-partition sum(d2) (acc2);
    # [0:16, 16] = per-channel sum(t); [0:16, 17] = per-channel sum(d2*t).
    out_all = nc.dram_tensor("out_all", [P, C + 2], f32, kind="ExternalOutput")

    from contextlib import ExitStack

    with ExitStack() as ctx:
        ctx.enter_context(nc.cleanup_on_exit())
        sb = lambda name, shape, dtype: ctx.enter_context(  # noqa: E731
            nc.sbuf_tensor(name, shape, dtype)
        )
        t_sb = {}
        n_sb = {}
        for g, chans in enumerate(GROUPS):
            t_sb[g] = sb(f"t_sb{g}", [P, len(chans), F], f16)
            n_sb[g] = sb(f"n_sb{g}", [P, len(chans), F], f16)
        d_sb = [sb(f"d_sb{k}", [P, F], f16) for k in range(RING)]
        d2_sb = [sb(f"d2_sb{k}", [P, F], f16) for k in range(RING)]
        p_sb = [sb(f"p_sb{k}", [P, F], f16) for k in range(RING)]
        oneh = sb("oneh_sb", [P, C, 16], f16)
        outb = sb("outb_sb", [P, C + 2], f32)
        scratch = sb("scratch_sb", [P, 1], f16)
        red_scr = sb("red_scr_sb", [16, 512], f32)
        psum1 = ctx.enter_context(nc.psum_tensor("psum1", [16, 512], f32))
        psum3 = ctx.enter_context(nc.psum_tensor("psum3", [16, 512], f32))

        sem = nc.alloc_semaphore
        s_t = [sem(f"s_t{g}") for g in range(NG)]
        s_n = [sem(f"s_n{g}") for g in range(NG)]
        s_oneh = sem("s_oneh")
        s_d = sem("s_d")      # subs completed
        s_sq = sem("s_sq")    # squares completed
        s_p = sem("s_p")      # muls completed
        s_pet = sem("s_pet")  # PE t-matmul channel groups completed
        s_pep = sem("s_pep")  # PE p-matmul channel groups completed
        s_red = sem("s_red")  # final reductions completed
        s_out = sem("s_out")  # output DMA completed

        def t_ap(c):
            g = grp_of[c]
            return t_sb[g][:, c - GROUPS[g][0], :]

        def n_ap(c):
            g = grp_of[c]
            return n_sb[g][:, c - GROUPS[g][0], :]

        # ---- GPSIMD: build one-hot weights on device (no DMA needed) ----
        nc.gpsimd.memset(oneh[:, :, :], 0.0)
        for c in range(C):
            ms = nc.gpsimd.memset(oneh[:, c, c : c + 1], 1.0)
        ms.then_inc(s_oneh, 1)

        # ---- SP: DMAs ----
        def in_dma(dst, src, chans, s):
            c0 = chans[0]
            nc.sync.dma_start(
                dst[:, :, :],
                src[c0 : c0 + len(chans)].rearrange("c p f -> p c f"),
            ).then_inc(s, 16)

        for g in range(NG):
            in_dma(t_sb[g], t_in.ap(), GROUPS[g], s_t[g])
            in_dma(n_sb[g], n_in.ap(), GROUPS[g], s_n[g])
        # acc2 columns ship as soon as the squares finish (overlaps the
        # final muls/matmuls); the tiny reduction outputs ship last.
        nc.sync.wait_ge(s_sq, C)
        nc.sync.dma_start(
            out_all.ap()[:, 0:C], outb[:, 0:C]
        ).then_inc(s_out, 16)
        nc.sync.wait_ge(s_red, 2)
        nc.sync.dma_start(
            out_all.ap()[0:16, C : C + 2], outb[0:16, C : C + 2]
        ).then_inc(s_out, 16)
        nc.sync.wait_ge(s_out, 32)

        # ---- DVE: subs (3 channels ahead) and muls ----
        def emit_sub(c):
            g = grp_of[c]
            if c == GROUPS[g][0]:
                nc.vector.wait_ge(s_t[g], 16)
                nc.vector.wait_ge(s_n[g], 16)
            nc.vector.tensor_tensor(
                d_sb[c % RING][:, :], t_ap(c), n_ap(c), Alu.subtract
            ).then_inc(s_d, 1)

        def emit_mul(j):
            nc.vector.wait_ge(s_sq, j + 1)
            if j >= RING:
                nc.vector.wait_ge(s_pep, j - (RING - 1))
            nc.vector.tensor_tensor(
                p_sb[j % RING][:, :], d2_sb[j % RING][:, :], t_ap(j), Alu.mult
            ).then_inc(s_p, 1)

        SKEW = 2
        for i in range(C + SKEW):
            if i < C:
                emit_sub(i)
            if i - SKEW >= 0:
                emit_mul(i - SKEW)

        # ---- ACT: squares with fused per-partition accumulation ----
        # Dummy activation first: pulls the one-time ACT_TABLE_LOAD
        # (~1.3us) off the critical path of the first real square.
        nc.scalar.activation(scratch[:, :], scratch[:, :], Act.Square)
        for c in range(C):
            nc.scalar.wait_ge(s_d, c + 1)
            if c >= RING:
                nc.scalar.wait_ge(s_p, c - (RING - 1))
            nc.scalar.activation(
                d2_sb[c % RING][:, :],
                d_sb[c % RING][:, :],
                Act.Square,
                accum_out=outb[:, c : c + 1],
            ).then_inc(s_sq, 1)
        # Final PSUM -> [16,1] reductions.
        if os.environ.get("BASS_REDUCE_ENGINE", "dve") == "act":
            nc.scalar.wait_ge(s_pet, C)
            nc.scalar.activation(
                red_scr[:, :], psum1[:, :], Act.Copy,
                accum_out=outb[0:16, C : C + 1],
            ).then_inc(s_red, 1)
            nc.scalar.wait_ge(s_pep, C)
            nc.scalar.activation(
                red_scr[:, :], psum3[:, :], Act.Copy,
                accum_out=outb[0:16, C + 1 : C + 2],
            ).then_inc(s_red, 1)
        else:
            nc.vector.wait_ge(s_pet, C)
            nc.vector.tensor_reduce(
                outb[0:16, C : C + 1], psum1[:, :],
                axis=mybir.AxisListType.X, op=Alu.add,
            ).then_inc(s_red, 1)
            nc.vector.wait_ge(s_pep, C)
            nc.vector.tensor_reduce(
                outb[0:16, C + 1 : C + 2], psum3[:, :],
                axis=mybir.AxisListType.X, op=Alu.add,
            ).then_inc(s_red, 1)

        # ---- PE: one-hot column-sum matmuls; t leads p by one group ----
        def emit_t_mms(g):
            nc.tensor.wait_ge(s_t[g], 16)
            for lc, c in enumerate(GROUPS[g]):
                w = oneh[:, c, :]
                off = 0
                for wdt in CHUNKS:
                    mm = nc.tensor.matmul(
                        psum1[:, 0:wdt],
                        lhsT=w,
                        rhs=t_sb[g][:, lc, off : off + wdt],
                        start=(c == 0 and off == 0),
                        stop=(c == C - 1 and off + wdt == F),
                        skip_group_check=True,
                    )
                    off += wdt
                mm.then_inc(s_pet, 1)

        def emit_p_mms(c):
            nc.tensor.wait_ge(s_p, c + 1)
            w = oneh[:, c, :]
            off = 0
            for wdt in CHUNKS:
                mm = nc.tensor.matmul(
                    psum3[:, 0:wdt],
                    lhsT=w,
                    rhs=p_sb[c % RING][:, off : off + wdt],
                    start=(c == 0 and off == 0),
                    stop=(c == C - 1 and off + wdt == F),
                    skip_group_check=True,
                )
                off += wdt
            mm.then_inc(s_pep, 1)

        nc.tensor.wait_ge(s_oneh, 1)
        emit_t_mms(0)
        done_p = 0
        for g in range(1, NG):
            emit_t_mms(g)
            for c in GROUPS[g - 1]:
                emit_p_mms(c)
                done_p = c
        for c in range(done_p + 1, C):
            emit_p_mms(c)

        nc.all_engine_barrier()

    return nc


def _get_nc():
    impl = os.environ.get("BASS_LOSS_IMPL", "raw")
    key = "nc_{}_{}_{}".format(
        impl,
        os.environ.get("BASS_HEAD_SINGLES", "4"),
        os.environ.get("BASS_REDUCE_ENGINE", "dve"),
    )
    if key not in _CACHE:
        _CACHE[key] = _build_bass_raw() if impl == "raw" else _build_bass()
    return _CACHE[key]


def kernel(net_out, target, max_positiones):
    from concourse import bass_utils

    nc = _get_nc()

    t16 = np.asarray(target, dtype=np.float16).reshape(B, C, P, F)
    n16 = np.asarray(net_out, dtype=np.float16).reshape(B, C, P, F)

    expected_inputs = set()
    import concourse.mybir as mybir

    for alloc in nc.m.functions[0].allocations:
        if (
            isinstance(alloc, mybir.MemoryLocationSet)
            and alloc.kind == "ExternalInput"
        ):
            expected_inputs.add(alloc.memorylocations[0].name)

    def make_map(b):
        m = {"t_in": t16[b], "n_in": n16[b]}
        if "oneh" in expected_inputs:
            oneh = np.zeros((P, C, 16), dtype=np.float16)
            for c in range(C):
                oneh[:, c, c] = 1.0
            m["oneh"] = oneh
        return m

    in_maps = [make_map(b) for b in range(B)]
    # The axon terminal occasionally reports the accelerator unrecoverable
    # on the first touch after a previous process ran a NEFF. The failed
    # attempt triggers recovery terminal-side, but the local PJRT client
    # stays poisoned — tear it down between retries.
    last_err = None
    for _attempt in range(4):
        try:
            res = bass_utils.run_bass_kernel_spmd(
                nc, in_maps, core_ids=list(range(8))
            )
            break
        except Exception as e:  # noqa: BLE001
            last_err = e
            import time as _time

            _time.sleep(3.0)
            try:
                import jax

                jax.clear_caches()
                jax.extend.backend.clear_backends()
            except Exception:  # noqa: BLE001
                pass
            _time.sleep(2.0)
    else:
        raise last_err

    S1 = np.empty((B, C), np.float64)
    S2 = np.empty((B, C), np.float64)
    S3 = np.empty((B, C), np.float64)
    for b in range(B):
        r = res.results[b]
        if "out_all" in r:
            out = r["out_all"].astype(np.float64)
            S1[b] = out[:16, C]
            S3[b] = out[:16, C + 1]
            S2[b] = out[:, :C].sum(axis=0)
        else:
            s13 = r["out_s13"].astype(np.float64)
            S1[b] = s13[:, 0]
            S3[b] = s13[:, 1]
            S2[b] = r["out_acc2"].astype(np.float64).sum(axis=0)

    m1, m2, d1 = S3, S2 - S3, S1
    d2n = float(HWE) - d1
    loss = ALPHA * m1 / (d1 + SMOOTH) + (1.0 - ALPHA) * m2 / (d2n + SMOOTH)

    # active-mask: S1 != 0 implies max(target[b,c]) != 0 for non-negative
    # targets; the S1 == 0 corner is resolved exactly on host.
    active = S1 != 0.0
    for b, c in zip(*np.nonzero(~active)):
        mt = np.max(target[b, c])
        mmp = np.max(max_positiones[b, c])
        active[b, c] = not (mt == 0.0 and mmp == 0.0)

    losses = np.where(active, loss, 0.0)
    count = (losses != 0.0).sum(axis=1).astype(np.float64)
    img_losses = losses.sum(axis=1) / count
    return np.float32(img_losses.mean())


# revision 41
# speedup vs baseline: 1.0754x; 1.0039x over previous
"""Trainium2 Bass kernel for nn_Mismatch_loss (weighted per-channel MSE loss).

Contract: kernel(**inputs) takes FULL fp32 inputs (net_out, target,
max_positiones of shape [8, 16, 384, 384]) and returns the FULL scalar
output, distributing work across 8 NeuronCores internally.

Sharding: data-parallel over batch — core b processes image b.

Math per (b, c) channel (spatial reductions over 384*384 = HW elements):
    d   = t - n
    d2  = d * d
    S1  = sum(t)        (= d1 in the reference)
    S2  = sum(d2)       (= m1 + m2)
    S3  = sum(d2 * t)   (= m1)
    loss = ALPHA*S3/(S1+eps) + (1-ALPHA)*(S2-S3)/(HW-S1+eps)
The tiny [B, C] -> scalar finalization (active-mask, count of nonzero
losses, means) runs on host from the gathered per-channel sums.

Device layout per core: channel c is a [128, 1152] tile (partition-major
split of the 147456 spatial elements). Engines:
  - DVE: d = t - n, p = d2 * t      (fp16 tensor_tensor, 2x mode)
  - ACT: d2 = Square(d) with accum_out -> per-partition sum(d2) columns
  - PE : per-channel column sums of t and p via one-hot fp16 weights,
         accumulated across chunks/channels into PSUM [16, 512]
  - fp32 accumulation everywhere (PSUM / accum_out are fp32)

Inputs are cast to fp16 on host before upload: halves HBM traffic (the
kernel is DMA-bound) at ~1e-5 relative error on the final scalar.

max_positiones is only consulted when a channel of target is exactly
all-zero (cannot happen for this problem's random-uniform inputs); that
case is handled exactly on host without shipping the tensor to devices.
"""

import os
import sys

import numpy as np

for _p in ("/opt/trn_rl_repo", "/root/.axon_site/_ro/trn_rl_repo"):
    if os.path.isdir(_p) and _p not in sys.path:
        sys.path.append(_p)

B, C, H, W = 8, 16, 384, 384
HWE = H * W          # 147456 spatial elements per channel
P = 128              # SBUF partitions
F = HWE // P         # 1152 elements per partition per channel
MACRO = 4            # channels per macro tile (per DMA)
N_MACRO = C // MACRO
CHUNKS = (512, 512, 128)   # PE matmul free-dim chunking of F
SMOOTH = 1e-6
ALPHA = 0.05

_CACHE = {}


def _build_bass():
    import concourse.bacc as bacc
    import concourse.mybir as mybir
    from concourse.tile import TileContext

    f16 = mybir.dt.float16
    f32 = mybir.dt.float32
    Alu = mybir.AluOpType

    # num_devices=1: the 8 cores run fully independent SPMD instances (no
    # collectives), so no cross-core barriers are needed.
    nc = bacc.Bacc(
        "TRN2", target_bir_lowering=False, debug=False, num_devices=1
    )
    t_in = nc.dram_tensor("t_in", [C, P, F], f16, kind="ExternalInput")
    n_in = nc.dram_tensor("n_in", [C, P, F], f16, kind="ExternalInput")
    # oneh[p, c, m] = 1.0 where m == c: stationary weights routing channel
    # c's column sums to PSUM partition c.
    oneh_in = nc.dram_tensor("oneh", [P, C, 16], f16, kind="ExternalInput")
    out_s13 = nc.dram_tensor("out_s13", [16, 2], f32, kind="ExternalOutput")
    out_acc2 = nc.dram_tensor("out_acc2", [P, C], f32, kind="ExternalOutput")

    with TileContext(nc) as tc:
        with (
            tc.tile_pool(name="io", bufs=N_MACRO) as io_pool,
            tc.tile_pool(name="cpool", bufs=6) as ch_pool,
            tc.tile_pool(name="consts", bufs=1) as const_pool,
            tc.tile_pool(name="accs", bufs=1) as acc_pool,
            tc.tile_pool(name="ps", bufs=1, space="PSUM") as psum_pool,
        ):
            oneh = const_pool.tile([P, C, 16], f16)
            nc.sync.dma_start(oneh, oneh_in.ap())
            acc2 = acc_pool.tile([P, C], f32)     # per-partition sum(d2), col c
            s13 = acc_pool.tile([16, 2], f32)
            psum1 = psum_pool.tile([16, 512], f32)  # sum(t) partials
            psum3 = psum_pool.tile([16, 512], f32)  # sum(d2*t) partials

            # Prefetch everything: all input DMAs issue up front, so the
            # SDMA engines stream continuously at HBM rate.
            t_tiles, n_tiles = [], []
            for m in range(N_MACRO):
                c0 = m * MACRO
                t_t = io_pool.tile([P, MACRO, F], f16, tag="t")
                n_t = io_pool.tile([P, MACRO, F], f16, tag="n")
                nc.sync.dma_start(
                    t_t, t_in.ap()[c0 : c0 + MACRO].rearrange("c p f -> p c f")
                )
                nc.sync.dma_start(
                    n_t, n_in.ap()[c0 : c0 + MACRO].rearrange("c p f -> p c f")
                )
                t_tiles.append(t_t)
                n_tiles.append(n_t)

            for m in range(N_MACRO):
                c0 = m * MACRO
                t_t, n_t = t_tiles[m], n_tiles[m]
                for lc in range(MACRO):
                    c = c0 + lc
                    d_c = ch_pool.tile([P, F], f16, tag="d")
                    nc.vector.tensor_tensor(
                        d_c, t_t[:, lc, :], n_t[:, lc, :], Alu.subtract
                    )
                    d2_c = ch_pool.tile([P, F], f16, tag="d2")
                    nc.scalar.activation(
                        d2_c,
                        d_c,
                        mybir.ActivationFunctionType.Square,
                        accum_out=acc2[:, c : c + 1],
                    )
                    p_c = ch_pool.tile([P, F], f16, tag="p")
                    nc.vector.tensor_tensor(p_c, d2_c, t_t[:, lc, :], Alu.mult)
                    w = oneh[:, c, :]
                    off = 0
                    for wdt in CHUNKS:
                        first = c == 0 and off == 0
                        last = c == C - 1 and off + wdt == F
                        nc.tensor.matmul(
                            psum1[:, 0:wdt],
                            lhsT=w,
                            rhs=t_t[:, lc, off : off + wdt],
                            start=first,
                            stop=last,
                            skip_group_check=True,
                        )
                        nc.tensor.matmul(
                            psum3[:, 0:wdt],
                            lhsT=w,
                            rhs=p_c[:, off : off + wdt],
                            start=first,
                            stop=last,
                            skip_group_check=True,
                        )
                        off += wdt

            nc.vector.tensor_reduce(
                s13[:, 0:1], psum1, axis=mybir.AxisListType.X, op=Alu.add
            )
            nc.vector.tensor_reduce(
                s13[:, 1:2], psum3, axis=mybir.AxisListType.X, op=Alu.add
            )
            nc.sync.dma_start(out_s13.ap(), s13)
            nc.sync.dma_start(out_acc2.ap(), acc2)

    nc.compile()
    return nc


def _build_bass_raw():
    """Hand-scheduled raw-bass version: same pipeline as the Tile build but
    with manual semaphores and a minimal end-of-kernel protocol, avoiding
    Tile's ~15us of preamble/postamble barriers.

    Engine programs (per core):
      SP  : all input DMAs up front (2-channel granularity), output DMAs
            gated on completion sems.
      DVE : d_c = t_c - n_c and p_c = d2_c * t_c (fp16 2x), subs running
            3 channels ahead of muls; final PSUM->[16,1] reductions.
      ACT : d2_c = Square(d_c) with fused per-partition accumulation.
      PE  : per-channel column sums of t and p into PSUM via one-hot
            weights, t-matmuls leading p-matmuls by one macro.
    """
    import concourse.bass as bass
    import concourse.mybir as mybir

    f16 = mybir.dt.float16
    f32 = mybir.dt.float32
    Alu = mybir.AluOpType
    Act = mybir.ActivationFunctionType

    # Channel groups per input DMA: first channels in 1-channel DMAs so
    # compute starts as early as possible, last channels in 1-channel DMAs
    # so the end-of-stream dependency tail is short, 2-channel in between.
    n_singles = int(os.environ.get("BASS_HEAD_SINGLES", "4"))
    GROUPS = [[c] for c in range(n_singles)] + [
        [c, c + 1] for c in range(n_singles, C, 2)
    ]
    NG = len(GROUPS)
    grp_of = {}
    for g, chans in enumerate(GROUPS):
        for c in chans:
            grp_of[c] = g
    RING = 4                     # d/d2/p ring depth (channels in flight)

    nc = bass.Bass("TRN2", target_bir_lowering=False, debug=False, num_devices=1)
    t_in = nc.dram_tensor("t_in", [C, P, F], f16, kind="ExternalInput")
    n_in = nc.dram_tensor("n_in", [C, P, F], f16, kind="ExternalInput")
    # Single merged output: cols 0..15 = per-partition sum(d2) (acc2);
    # [0:16, 16] = per-channel sum(t); [0:16, 17] = per-channel sum(d2*t).
    out_all = nc.dram_tensor("out_all", [P, C + 2], f32, kind="ExternalOutput")

    from contextlib import ExitStack

    with ExitStack() as ctx:
        ctx.enter_context(nc.cleanup_on_exit())
        sb = lambda name, shape, dtype: ctx.enter_context(  # noqa: E731
            nc.sbuf_tensor(name, shape, dtype)
        )
        t_sb = {}
        n_sb = {}
        for g, chans in enumerate(GROUPS):
            t_sb[g] = sb(f"t_sb{g}", [P, len(chans), F], f16)
            n_sb[g] = sb(f"n_sb{g}", [P, len(chans), F], f16)
        d_sb = [sb(f"d_sb{k}", [P, F], f16) for k in range(RING)]
        d2_sb = [sb(f"d2_sb{k}", [P, F], f16) for k in range(RING)]
        p_sb = [sb(f"p_sb{k}", [P, F], f16) for k in range(RING)]
        oneh = sb("oneh_sb", [P, C, 16], f16)
        outb = sb("outb_sb", [P, C + 2], f32)
        scratch = sb("scratch_sb", [P, 1], f16)
        red_scr = sb("red_scr_sb", [16, 512], f32)
        psum1 = ctx.enter_context(nc.psum_tensor("psum1", [16, 512], f32))
        psum3 = ctx.enter_context(nc.psum_tensor("psum3", [16, 512], f32))

        sem = nc.alloc_semaphore
        s_t = [sem(f"s_t{g}") for g in range(NG)]
        s_n = [sem(f"s_n{g}") for g in range(NG)]
        s_oneh = sem("s_oneh")
        s_d = sem("s_d")      # subs completed
        s_sq = sem("s_sq")    # squares completed
        s_p = sem("s_p")      # muls completed
        s_pet = sem("s_pet")  # PE t-matmul channel groups completed
        s_pep = sem("s_pep")  # PE p-matmul channel groups completed
        s_red = sem("s_red")  # final reductions completed
        s_out = sem("s_out")  # output DMA completed

        def t_ap(c):
            g = grp_of[c]
            return t_sb[g][:, c - GROUPS[g][0], :]

        def n_ap(c):
            g = grp_of[c]
            return n_sb[g][:, c - GROUPS[g][0], :]

        # ---- GPSIMD: build one-hot weights on device (no DMA needed) ----
        nc.gpsimd.memset(oneh[:, :, :], 0.0)
        for c in range(C):
            ms = nc.gpsimd.memset(oneh[:, c, c : c + 1], 1.0)
        ms.then_inc(s_oneh, 1)

        # ---- SP: DMAs ----
        def in_dma(dst, src, chans, s):
            c0 = chans[0]
            nc.sync.dma_start(
                dst[:, :, :],
                src[c0 : c0 + len(chans)].rearrange("c p f -> p c f"),
            ).then_inc(s, 16)

        for g in range(NG):
            in_dma(t_sb[g], t_in.ap(), GROUPS[g], s_t[g])
            in_dma(n_sb[g], n_in.ap(), GROUPS[g], s_n[g])
        # acc2 columns ship as soon as the squares finish (overlaps the
        # final muls/matmuls); the tiny reduction outputs ship last.
        nc.sync.wait_ge(s_sq, C)
        nc.sync.dma_start(
            out_all.ap()[:, 0:C], outb[:, 0:C]
        ).then_inc(s_out, 16)
        nc.sync.wait_ge(s_red, 2)
        nc.sync.dma_start(
            out_all.ap()[0:16, C : C + 2], outb[0:16, C : C + 2]
        ).then_inc(s_out, 16)
        nc.sync.wait_ge(s_out, 32)

        # ---- DVE: subs (3 channels ahead) and muls ----
        def emit_sub(c):
            g = grp_of[c]
            if c == GROUPS[g][0]:
                nc.vector.wait_ge(s_t[g], 16)
                nc.vector.wait_ge(s_n[g], 16)
            nc.vector.tensor_tensor(
                d_sb[c % RING][:, :], t_ap(c), n_ap(c), Alu.subtract
            ).then_inc(s_d, 1)

        def emit_mul(j):
            nc.vector.wait_ge(s_sq, j + 1)
            if j >= RING:
                nc.vector.wait_ge(s_pep, j - (RING - 1))
            nc.vector.tensor_tensor(
                p_sb[j % RING][:, :], d2_sb[j % RING][:, :], t_ap(j), Alu.mult
            ).then_inc(s_p, 1)

        SKEW = 2
        for i in range(C + SKEW):
            if i < C:
                emit_sub(i)
            if i - SKEW >= 0:
                emit_mul(i - SKEW)

        # ---- ACT: squares with fused per-partition accumulation ----
        # Dummy activation first: pulls the one-time ACT_TABLE_LOAD
        # (~1.3us) off the critical path of the first real square.
        nc.scalar.activation(scratch[:, :], scratch[:, :], Act.Square)
        for c in range(C):
            nc.scalar.wait_ge(s_d, c + 1)
            if c >= RING:
                nc.scalar.wait_ge(s_p, c - (RING - 1))
            nc.scalar.activation(
                d2_sb[c % RING][:, :],
                d_sb[c % RING][:, :],
                Act.Square,
                accum_out=outb[:, c : c + 1],
            ).then_inc(s_sq, 1)
        # Final PSUM -> [16,1] reductions.
        if os.environ.get("BASS_REDUCE_ENGINE", "act") == "act":
            nc.scalar.wait_ge(s_pet, C)
            nc.scalar.activation(
                red_scr[:, :], psum1[:, :], Act.Copy,
                accum_out=outb[0:16, C : C + 1],
            ).then_inc(s_red, 1)
            nc.scalar.wait_ge(s_pep, C)
            nc.scalar.activation(
                red_scr[:, :], psum3[:, :], Act.Copy,
                accum_out=outb[0:16, C + 1 : C + 2],
            ).then_inc(s_red, 1)
        else:
            nc.vector.wait_ge(s_pet, C)
            nc.vector.tensor_reduce(
                outb[0:16, C : C + 1], psum1[:, :],
                axis=mybir.AxisListType.X, op=Alu.add,
            ).then_inc(s_red, 1)
            nc.vector.wait_ge(s_pep, C)
            nc.vector.tensor_reduce(
                outb[0:16, C + 1 : C + 2], psum3[:, :],
                axis=mybir.AxisListType.X, op=Alu.add,
            ).then_inc(s_red, 1)

        # ---- PE: one-hot column-sum matmuls; t leads p by one group ----
        def emit_t_mms(g):
            nc.tensor.wait_ge(s_t[g], 16)
            for lc, c in enumerate(GROUPS[g]):
                w = oneh[:, c, :]
                off = 0
                for wdt in CHUNKS:
                    mm = nc.tensor.matmul(
                        psum1[:, 0:wdt],
                        lhsT=w,
                        rhs=t_sb[g][:, lc, off : off + wdt],
                        start=(c == 0 and off == 0),
                        stop=(c == C - 1 and off + wdt == F),
                        skip_group_check=True,
                    )
                    off += wdt
                mm.then_inc(s_pet, 1)

        def emit_p_mms(c):
            nc.tensor.wait_ge(s_p, c + 1)
            w = oneh[:, c, :]
            off = 0
            for wdt in CHUNKS:
                mm = nc.tensor.matmul(
                    psum3[:, 0:wdt],
                    lhsT=w,
                    rhs=p_sb[c % RING][:, off : off + wdt],
                    start=(c == 0 and off == 0),
                    stop=(c == C - 1 and off + wdt == F),
                    skip_group_check=True,
                )
                off += wdt
            mm.then_inc(s_pep, 1)

        nc.tensor.wait_ge(s_oneh, 1)
        emit_t_mms(0)
        done_p = 0
        for g in range(1, NG):
            emit_t_mms(g)
            for c in GROUPS[g - 1]:
                emit_p_mms(c)
                done_p = c
        for c in range(done_p + 1, C):
            emit_p_mms(c)

        nc.all_engine_barrier()

    return nc


def _get_nc():
    impl = os.environ.get("BASS_LOSS_IMPL", "raw")
    key = "nc_{}_{}_{}".format(
        impl,
        os.environ.get("BASS_HEAD_SINGLES", "4"),
        os.environ.get("BASS_REDUCE_ENGINE", "act"),
    )
    if key not in _CACHE:
        _CACHE[key] = _build_bass_raw() if impl == "raw" else _build_bass()
    return _CACHE[key]


def kernel(net_out, target, max_positiones):
    from concourse import bass_utils

    nc = _get_nc()

    t16 = np.asarray(target, dtype=np.float16).reshape(B, C, P, F)
    n16 = np.asarray(net_out, dtype=np.float16).reshape(B, C, P, F)

    expected_inputs = set()
    import concourse.mybir as mybir

    for alloc in nc.m.functions[0].allocations:
        if (
            isinstance(alloc, mybir.MemoryLocationSet)
            and alloc.kind == "ExternalInput"
        ):
            expected_inputs.add(alloc.memorylocations[0].name)

    def make_map(b):
        m = {"t_in": t16[b], "n_in": n16[b]}
        if "oneh" in expected_inputs:
            oneh = np.zeros((P, C, 16), dtype=np.float16)
            for c in range(C):
                oneh[:, c, c] = 1.0
            m["oneh"] = oneh
        return m

    in_maps = [make_map(b) for b in range(B)]
    # The axon terminal occasionally reports the accelerator unrecoverable
    # on the first touch after a previous process ran a NEFF. The failed
    # attempt triggers recovery terminal-side, but the local PJRT client
    # stays poisoned — tear it down between retries.
    last_err = None
    for _attempt in range(4):
        try:
            res = bass_utils.run_bass_kernel_spmd(
                nc, in_maps, core_ids=list(range(8))
            )
            break
        except Exception as e:  # noqa: BLE001
            last_err = e
            import time as _time

            _time.sleep(3.0)
            try:
                import jax

                jax.clear_caches()
                jax.extend.backend.clear_backends()
            except Exception:  # noqa: BLE001
                pass
            _time.sleep(2.0)
    else:
        raise last_err

    S1 = np.empty((B, C), np.float64)
    S2 = np.empty((B, C), np.float64)
    S3 = np.empty((B, C), np.float64)
    for b in range(B):
        r = res.results[b]
        if "out_all" in r:
            out = r["out_all"].astype(np.float64)
            S1[b] = out[:16, C]
            S3[b] = out[:16, C + 1]
            S2[b] = out[:, :C].sum(axis=0)
        else:
            s13 = r["out_s13"].astype(np.float64)
            S1[b] = s13[:, 0]
            S3[b] = s13[:, 1]
            S2[b] = r["out_acc2"].astype(np.float64).sum(axis=0)

    m1, m2, d1 = S3, S2 - S3, S1
    d2n = float(HWE) - d1
    loss = ALPHA * m1 / (d1 + SMOOTH) + (1.0 - ALPHA) * m2 / (d2n + SMOOTH)

    # active-mask: S1 != 0 implies max(target[b,c]) != 0 for non-negative
    # targets; the S1 == 0 corner is resolved exactly on host.
    active = S1 != 0.0
    for b, c in zip(*np.nonzero(~active)):
        mt = np.max(target[b, c])
        mmp = np.max(max_positiones[b, c])
        active[b, c] = not (mt == 0.0 and mmp == 0.0)

    losses = np.where(active, loss, 0.0)
    count = (losses != 0.0).sum(axis=1).astype(np.float64)
    img_losses = losses.sum(axis=1) / count
    return np.float32(img_losses.mean())
